# revision 38
# baseline (speedup 1.0000x reference)
"""Trainium2 Bass kernel for CrispComposition.

Computes out[b, i] = max_o( min(m[b, i], weight[i, o]) ).

Since min(m, .) is monotone non-decreasing, the max over o commutes with it:
    max_o min(m, w[i, o]) = min(m, max_o w[i, o])
so the kernel reduces weight over its OUT axis once (wmax[i] = max_o
weight[i, o]) and streams an elementwise min over m. All min/max compute
runs on device; the host only stages layout (transpose/cast/shard).

Precision: inputs are cast to bf16 host-side and the output is returned as
bf16 upcast to f32. Each output element is min(bf16(m), bf16(wmax)) — a bf16
rounding of one of the original inputs (max/min select values, they don't
create new ones), so relative error <= 2^-9 ~= 2e-3, inside the 2e-2 gate.

Sharding: by the IN axis: core c owns IN rows [64c, 64c+64) for ALL 4096
batch samples. Each core needs only ITS 64 rows of weight (replicated twice
across the 128 partitions -> [128, 256], 64KB) instead of the full
replicated weight (256KB), and wmax falls out of one reduce_max. m is
staged host-side as [128, 2048] bf16 per core: partition p holds IN row
64c + (p % 64), batch half p // 64. The elementwise min is a per-partition
tensor_scalar_min against wmax[128, 1].

Schedule (tuned against the TRN2 instruction cost model):
  - The weight rides in ONE leading SP DMA together with the first m
    columns ("wm") so a single DMA-completion sem (+900ns prop) gates both
    the wmax reduce and the first min chunk.
  - Remaining m columns load as plain chunked DMAs (SP / Act HWDGE or Pool
    SWDGE), sized so the chunk needed LAST is small.
  - Stores use prepared SWDGE scatter writes: dma_scatter_add descriptors
    are generated EARLY (prepare_only on a dedicated SWDGE queue per
    chunk, identity iota indices, output pre-zeroed by the runtime so
    += is a plain write), and each chunk's trigger_dma fires right after
    its tensor_scalar_min lands — replacing the ~1.4us HWDGE store-issue
    path (SEQ+descgen+DGE delay) with a ~60ns Pool trigger.
"""

import numpy as np
import ml_dtypes

import concourse.bacc as bacc
import concourse.mybir as mybir
from concourse.bass_utils import run_bass_kernel_spmd

from concourse.tile import TileContext, add_dep_helper

B, IN, OUT = 4096, 512, 256
NCORES = 8
RPC = IN // NCORES  # 64 IN rows per core
P = 128  # SBUF partitions
COLS = B * RPC // P  # 2048 free-dim columns per core (batch folded)
OUT_ROWS_PAD = 256  # out DRAM rows padded: stray iota idx values (<=239)
# must stay below the row count for the scatter bounds assert

BF16 = mybir.dt.bfloat16
F32 = mybir.dt.float32
I16 = mybir.dt.int16

DEFAULT = dict(
    wm_mcols=512,
    loads=((512, "gpsimd"), (1024, "sync")),
    stores=(1024, 1024),
    trigger_mode="split",
)


def build_bass(
    wm_mcols=DEFAULT["wm_mcols"],
    loads=DEFAULT["loads"],
    stores=DEFAULT["stores"],
    trigger_mode=DEFAULT.get("trigger_mode", "single"),
):
    """wm_mcols: m columns bundled into the leading weight DMA.
    loads: (ncols, engine) for the remaining m columns.
    stores: store-chunk column counts (scatter-write chunks, <= 4).
    trigger_mode: "single" = all preps early, one trigger after the last
    min; "paired" = per chunk [prep_k, mins_k, trigger_k(count=None)] so
    chunk k's store fires as soon as its own min lands (official
    Tile-managed path — each trigger's pending list holds only its prep)."""
    assert sum(c for c, _ in loads) == COLS - wm_mcols
    assert sum(stores) == COLS
    assert len(stores) <= 4

    nc = bacc.Bacc()
    wm_in = nc.declare_dram_parameter("wm", [P, OUT + wm_mcols], BF16, isOutput=False)
    m_in = (
        nc.declare_dram_parameter("m", [P, COLS - wm_mcols], BF16, isOutput=False)
        if wm_mcols < COLS
        else None
    )
    out = nc.declare_dram_parameter("out", [OUT_ROWS_PAD, COLS], BF16, isOutput=True)

    eng = {"sync": nc.sync, "scalar": nc.scalar, "gpsimd": nc.gpsimd}

    with TileContext(nc) as tc:
        with (
            tc.tile_pool(name="consts", bufs=1) as consts,
            tc.tile_pool(name="wmpool", bufs=1) as wmpool,
            tc.tile_pool(name="mpool", bufs=max(1, len(loads))) as mpool,
            tc.tile_pool(name="opool", bufs=len(stores)) as opool,
        ):
            idx = consts.tile([P, 8], I16, tag="idx")
            wmt = wmpool.tile([P, OUT + wm_mcols], BF16, tag="wm")
            wmax = consts.tile([P, 1], F32, name="wmax", tag="wx")

            # identity scatter indices: idx[p, g] = p + 16g -> unwrapped[k]=k
            nc.gpsimd.iota(idx, pattern=[[16, 8]], base=0, channel_multiplier=1)

            # leading DMA: weight + first m columns, one completion sem
            nc.sync.dma_start(out=wmt, in_=wm_in[:, :])

            # m tiles indexed by absolute column range; the wm tile's m part
            # is the range [0, wm_mcols) at offset OUT
            mtiles = []
            if wm_mcols:
                mtiles.append((wmt, 0, wm_mcols, OUT))
            c0 = wm_mcols
            for ncols, e in loads:
                mt = mpool.tile([P, ncols], BF16, tag=f"m{c0}")
                eng[e].dma_start(
                    out=mt, in_=m_in[:, c0 - wm_mcols : c0 - wm_mcols + ncols]
                )
                mtiles.append((mt, c0, ncols, 0))
                c0 += ncols

            otiles = []
            c0 = 0
            for k, ncols in enumerate(stores):
                ot = opool.tile([P, 1, ncols], BF16, tag=f"o{c0}")
                otiles.append((ot, c0, ncols))
                c0 += ncols

            def emit_prep(k):
                ot, c0_, ncols = otiles[k]
                sem = nc.alloc_semaphore(f"sc_dma{k}")
                return nc.gpsimd.dma_scatter_add(
                    out[:, c0_ : c0_ + ncols],
                    ot[:, :, :],
                    idx[:, :],
                    P,
                    P,
                    ncols,
                    elem_step=COLS,
                    prepare_only=True,
                    sem=sem,
                )

            def emit_mins(k):
                ot, c0_, ncols = otiles[k]
                lo, hi = c0_, c0_ + ncols
                last = None
                for mt, mc0, mcols, moff in mtiles:
                    a, b = max(lo, mc0), min(hi, mc0 + mcols)
                    if a >= b:
                        continue
                    last = nc.vector.tensor_scalar_min(
                        out=ot[:, 0, a - lo : b - lo],
                        in0=mt[:, moff + a - mc0 : moff + b - mc0],
                        scalar1=wmax,
                    )
                return last

            if trigger_mode == "split":
                # Two chunks: both preps early, so chunk 1's desc-gen runs
                # during the loads instead of gating trigger 1. Trigger 0
                # (count=1) fires prep 0's ring entry; its inherited deps
                # are backward-looking, and chunk 1's mins are emitted
                # AFTER it, so it waits only chunk 0's min. Trigger 1
                # (count=1, ring FIFO -> prep 1's entry) gets its data dep
                # on chunk 1's last min explicitly (not elidable: that min
                # is not covered by trigger 0's clock).
                preps = []
                for k in range(len(stores)):
                    p = emit_prep(k)
                    if preps:
                        # pin the ring FIFO order — without this edge Tile
                        # may reorder the (independent) preps and trigger k
                        # would fire the WRONG chunk's entry
                        add_dep_helper(p.ins, preps[-1].ins, sync=False)
                    preps.append(p)
                nc.vector.reduce_max(
                    out=wmax, in_=wmt[:, :OUT], axis=mybir.AxisListType.X
                )
                # chunk 0's mins precede trigger 0, so its inherited
                # (backward-looking) deps cover exactly min0. Trigger 0
                # also inherits a wait on EVERY pending prep's engine tick
                # (it could legally fire any of them) — the explicit
                # wait_ge(prep_sem, k+1) before each trigger provides the
                # correct per-entry desc-gen gate, and patch_trig0_prep_wait
                # drops the over-broad inherited one.
                emit_mins(0)
                prev = nc.gpsimd.trigger_dma(count=1)
                for k in range(1, len(stores)):
                    mk = emit_mins(k)
                    tk = nc.gpsimd.trigger_dma(count=1)
                    add_dep_helper(tk.ins, mk.ins, sync=True)
                    add_dep_helper(tk.ins, prev.ins, sync=False)
                    prev = tk
            elif trigger_mode == "single":
                # all preps early; one trigger inherits every prep's
                # deferred source dep, waits the last min, then fires all
                # ring entries back-to-back
                for k in range(len(stores)):
                    emit_prep(k)
                nc.vector.reduce_max(
                    out=wmax, in_=wmt[:, :OUT], axis=mybir.AxisListType.X
                )
                for k in range(len(stores)):
                    emit_mins(k)
                nc.gpsimd.trigger_dma(count=None)
            else:
                # paired: trigger k fires right after chunk k's min. Each
                # trigger_dma(count=None) consumes exactly its own prep's
                # pending entry, so its inherited dep is just that chunk's
                # min. prep k+1's desc-gen runs behind trigger k on
                # Pool.SEQ (~1us), which gates trigger k+1 — keep chunk
                # counts low.
                emit_prep(0)
                nc.vector.reduce_max(
                    out=wmax, in_=wmt[:, :OUT], axis=mybir.AxisListType.X
                )
                emit_mins(0)
                nc.gpsimd.trigger_dma(count=None)
                for k in range(1, len(stores)):
                    emit_prep(k)
                    emit_mins(k)
                    nc.gpsimd.trigger_dma(count=None)

    return nc


def patch_orphan_dmasw_waits(nc):
    """gen_mode==1 SWDGE preps defer their DMA-completion sem (on_update[0],
    baked into the descriptor at prep time) to trigger time, but Tile's wait
    pass still emits consumer waits against the prep's round-robin DMASW
    lane sem, which nothing increments. Rewrite those orphaned waits to the
    prep's real completion sem (same >=16 semantics)."""
    fn = nc.m.functions[0]
    insts = [i for b in fn.blocks for i in b.instructions]
    incs = {}
    for inst in insts:
        si = inst.sync_info
        if not si:
            continue
        for u in si.on_update:
            incs[u.id] = incs.get(u.id, 0) + (u.update_value or 0)
    lane_sem = {}
    k = 0
    for inst in insts:
        tn = type(inst).__name__
        if inst.engine == mybir.EngineType.Pool and (
            "DMACopy" in tn or "Gather" in tn or "Scatter" in tn
            or "Writeback" in tn or "RemoteDMA" in tn
        ):
            lane = k % 8
            k += 1
            if getattr(inst, "gen_mode", 0) == 1:
                si = inst.sync_info
                assert si and len(si.on_update) >= 1
                u0 = si.on_update[0]
                assert lane not in lane_sem, "one prep per DMASW lane"
                lane_sem[lane] = (u0.id, u0.ant_name)
    n = 0
    for inst in insts:
        si = inst.sync_info
        if not si:
            continue
        for w in si.on_wait:
            if (
                w.ant_name
                and w.ant_name.startswith("DMASW")
                and incs.get(w.id, 0) < (w.wait_value or 0)
            ):
                lane = int(w.ant_name[5:].split("_")[0])
                nid, nname = lane_sem[lane]
                w.id = nid
                w.ant_name = nname
                n += 1
    return n


def patch_split_war_waits(nc):
    """trigger_mode="split" emits chunk 1's mins AFTER the trigger that
    consumed prep 1's pending entry, so Tile adds a WAR wait on the mins:
    writer-of-otile1 waits the prep's deferred read = the s1 DMA completion
    sem — circular (that DMA fires only after the min). The hazard it
    guards is already covered: trigger 1 has an explicit sync dep on the
    last min, so the scatter can never read otile1 before the mins wrote
    it. Drop the circular wait (DVE instructions only; the final drains'
    completion waits on the same sems must stay)."""
    fn = nc.m.functions[0]
    n = 0
    for b in fn.blocks:
        for inst in b.instructions:
            if inst.engine != mybir.EngineType.DVE:
                continue
            si = inst.sync_info
            if not si:
                continue
            for w in si.on_wait:
                if (w.ant_name or "").startswith("sc_dma") and (
                    w.wait_value or 0
                ) > 0:
                    w.wait_value = 0
                    n += 1
    return n


def patch_trig0_prep_wait(nc):
    """Drop the first trigger's inherited wait on the Pool engine sem (it
    covers ALL pending preps' desc-gen ticks; the per-trigger
    wait_ge(sc_prep, k+1) already gates each trigger on exactly its own
    prep's descriptor-write completion)."""
    fn = nc.m.functions[0]
    insts = [i for b in fn.blocks for i in b.instructions]
    # only when the explicit sc_prep gate exists (split mode emits
    # wait_ge(sc_prep, k+1) before every trigger); NOTE: dropping this wait
    # crashed real HW in the 3-chunk test — keep it disabled until the
    # explicit gate is proven equivalent on silicon.
    if not any(
        (w.ant_name or "") == "sc_prep"
        for i in insts
        if i.sync_info
        for w in i.sync_info.on_wait
    ):
        return 0
    return 0


_NC_CACHE = {}


def _get_nc(**kwargs):
    key = repr(sorted(kwargs.items()))
    if key not in _NC_CACHE:
        nc = build_bass(**kwargs)
        nc.finalize()
        patch_orphan_dmasw_waits(nc)
        patch_split_war_waits(nc)
        patch_trig0_prep_wait(nc)
        _NC_CACHE[key] = nc
    return _NC_CACHE[key]


def shard_inputs(m, weight, wm_mcols=DEFAULT["wm_mcols"]):
    """Host-side staging: cast to bf16, transpose m so IN is the partition
    axis, fold the batch axis into partitions (2 halves), shard by IN rows.
    The first wm_mcols m columns are concatenated onto the weight tile."""
    bf16 = ml_dtypes.bfloat16
    mT = np.asarray(m, dtype=bf16).T  # [IN, B]
    wb = np.asarray(weight, dtype=bf16)  # [IN, OUT]
    half = B // 2
    in_maps = []
    for c in range(NCORES):
        rows = mT[c * RPC : (c + 1) * RPC]  # [64, B]
        m_c = np.concatenate([rows[:, :half], rows[:, half:]], axis=0)
        w_c = np.tile(wb[c * RPC : (c + 1) * RPC, :], (2, 1))  # [128, 256]
        io = {
            "wm": np.ascontiguousarray(
                np.concatenate([w_c, m_c[:, :wm_mcols]], axis=1)
            )
        }
        if wm_mcols < COLS:
            io["m"] = np.ascontiguousarray(m_c[:, wm_mcols:])
        in_maps.append(io)
    return in_maps


def unshard_output(results):
    """Per-core [OUT_ROWS_PAD, 2048] bf16 (first 128 rows valid) ->
    full [B, IN] f32."""
    half = B // 2
    outT = np.empty((IN, B), dtype=np.float32)
    for c in range(NCORES):
        o_c = np.asarray(results[c]["out"])[:P]  # [128, 2048] bf16
        outT[c * RPC : (c + 1) * RPC, :half] = o_c[:RPC]
        outT[c * RPC : (c + 1) * RPC, half:] = o_c[RPC:]
    return np.ascontiguousarray(outT.T)


def run(m, weight, build_kwargs=None, **spmd_kwargs):
    """Run the bass kernel; returns (full_output, BassKernelResults)."""
    bk = dict(DEFAULT)
    bk.update(build_kwargs or {})
    nc = _get_nc(**bk)
    in_maps = shard_inputs(m, weight, wm_mcols=bk["wm_mcols"])
    res = run_bass_kernel_spmd(nc, in_maps, list(range(NCORES)), **spmd_kwargs)
    return unshard_output(res.results), res


def kernel(m, weight):
    return run(m, weight)[0]


# revision 39
# speedup vs baseline: 1.0063x; 1.0063x over previous
"""Trainium2 Bass kernel for CrispComposition.

Computes out[b, i] = max_o( min(m[b, i], weight[i, o]) ).

Since min(m, .) is monotone non-decreasing, the max over o commutes with it:
    max_o min(m, w[i, o]) = min(m, max_o w[i, o])
so the kernel reduces weight over its OUT axis once (wmax[i] = max_o
weight[i, o]) and streams an elementwise min over m. All min/max compute
runs on device; the host only stages layout (transpose/cast/shard).

Precision: inputs are cast to bf16 host-side and the output is returned as
bf16 upcast to f32. Each output element is min(bf16(m), bf16(wmax)) — a bf16
rounding of one of the original inputs (max/min select values, they don't
create new ones), so relative error <= 2^-9 ~= 2e-3, inside the 2e-2 gate.

Sharding: by the IN axis: core c owns IN rows [64c, 64c+64) for ALL 4096
batch samples. Each core needs only ITS 64 rows of weight (replicated twice
across the 128 partitions -> [128, 256], 64KB) instead of the full
replicated weight (256KB), and wmax falls out of one reduce_max. m is
staged host-side as [128, 2048] bf16 per core: partition p holds IN row
64c + (p % 64), batch half p // 64. The elementwise min is a per-partition
tensor_scalar_min against wmax[128, 1].

Schedule (tuned against the TRN2 instruction cost model):
  - The weight rides in ONE leading SP DMA together with the first m
    columns ("wm") so a single DMA-completion sem (+900ns prop) gates both
    the wmax reduce and the first min chunk.
  - Remaining m columns load as plain chunked DMAs (SP / Act HWDGE or Pool
    SWDGE), sized so the chunk needed LAST is small.
  - Stores use prepared SWDGE scatter writes: dma_scatter_add descriptors
    are generated EARLY (prepare_only on a dedicated SWDGE queue per
    chunk, identity iota indices, output pre-zeroed by the runtime so
    += is a plain write), and each chunk's trigger_dma fires right after
    its tensor_scalar_min lands — replacing the ~1.4us HWDGE store-issue
    path (SEQ+descgen+DGE delay) with a ~60ns Pool trigger.
"""

import numpy as np
import ml_dtypes

import concourse.bacc as bacc
import concourse.mybir as mybir
from concourse.bass_utils import run_bass_kernel_spmd

from concourse.tile import TileContext, add_dep_helper

B, IN, OUT = 4096, 512, 256
NCORES = 8
RPC = IN // NCORES  # 64 IN rows per core
P = 128  # SBUF partitions
COLS = B * RPC // P  # 2048 free-dim columns per core (batch folded)
OUT_ROWS_PAD = 256  # out DRAM rows padded: stray iota idx values (<=239)
# must stay below the row count for the scatter bounds assert

BF16 = mybir.dt.bfloat16
F32 = mybir.dt.float32
I16 = mybir.dt.int16

DEFAULT = dict(
    wm_mcols=448,
    loads=((640, "gpsimd"), (960, "sync")),
    stores=(1024, 1024),
    trigger_mode="split",
)


def build_bass(
    wm_mcols=DEFAULT["wm_mcols"],
    loads=DEFAULT["loads"],
    stores=DEFAULT["stores"],
    trigger_mode=DEFAULT.get("trigger_mode", "single"),
):
    """wm_mcols: m columns bundled into the leading weight DMA.
    loads: (ncols, engine) for the remaining m columns.
    stores: store-chunk column counts (scatter-write chunks, <= 4).
    trigger_mode: "single" = all preps early, one trigger after the last
    min; "paired" = per chunk [prep_k, mins_k, trigger_k(count=None)] so
    chunk k's store fires as soon as its own min lands (official
    Tile-managed path — each trigger's pending list holds only its prep)."""
    assert sum(c for c, _ in loads) == COLS - wm_mcols
    assert sum(stores) == COLS
    assert len(stores) <= 4

    nc = bacc.Bacc()
    wm_in = nc.declare_dram_parameter("wm", [P, OUT + wm_mcols], BF16, isOutput=False)
    m_in = (
        nc.declare_dram_parameter("m", [P, COLS - wm_mcols], BF16, isOutput=False)
        if wm_mcols < COLS
        else None
    )
    out = nc.declare_dram_parameter("out", [OUT_ROWS_PAD, COLS], BF16, isOutput=True)

    eng = {"sync": nc.sync, "scalar": nc.scalar, "gpsimd": nc.gpsimd}

    with TileContext(nc) as tc:
        with (
            tc.tile_pool(name="consts", bufs=1) as consts,
            tc.tile_pool(name="wmpool", bufs=1) as wmpool,
            tc.tile_pool(name="mpool", bufs=max(1, len(loads))) as mpool,
            tc.tile_pool(name="opool", bufs=len(stores)) as opool,
        ):
            idx = consts.tile([P, 8], I16, tag="idx")
            wmt = wmpool.tile([P, OUT + wm_mcols], BF16, tag="wm")
            wmax = consts.tile([P, 1], F32, name="wmax", tag="wx")

            # identity scatter indices: idx[p, g] = p + 16g -> unwrapped[k]=k
            nc.gpsimd.iota(idx, pattern=[[16, 8]], base=0, channel_multiplier=1)

            # leading DMA: weight + first m columns, one completion sem
            nc.sync.dma_start(out=wmt, in_=wm_in[:, :])

            # m tiles indexed by absolute column range; the wm tile's m part
            # is the range [0, wm_mcols) at offset OUT
            mtiles = []
            if wm_mcols:
                mtiles.append((wmt, 0, wm_mcols, OUT))
            c0 = wm_mcols
            for ncols, e in loads:
                mt = mpool.tile([P, ncols], BF16, tag=f"m{c0}")
                eng[e].dma_start(
                    out=mt, in_=m_in[:, c0 - wm_mcols : c0 - wm_mcols + ncols]
                )
                mtiles.append((mt, c0, ncols, 0))
                c0 += ncols

            otiles = []
            c0 = 0
            for k, ncols in enumerate(stores):
                ot = opool.tile([P, 1, ncols], BF16, tag=f"o{c0}")
                otiles.append((ot, c0, ncols))
                c0 += ncols

            def emit_prep(k):
                ot, c0_, ncols = otiles[k]
                sem = nc.alloc_semaphore(f"sc_dma{k}")
                return nc.gpsimd.dma_scatter_add(
                    out[:, c0_ : c0_ + ncols],
                    ot[:, :, :],
                    idx[:, :],
                    P,
                    P,
                    ncols,
                    elem_step=COLS,
                    prepare_only=True,
                    sem=sem,
                )

            def emit_mins(k):
                ot, c0_, ncols = otiles[k]
                lo, hi = c0_, c0_ + ncols
                last = None
                for mt, mc0, mcols, moff in mtiles:
                    a, b = max(lo, mc0), min(hi, mc0 + mcols)
                    if a >= b:
                        continue
                    last = nc.vector.tensor_scalar_min(
                        out=ot[:, 0, a - lo : b - lo],
                        in0=mt[:, moff + a - mc0 : moff + b - mc0],
                        scalar1=wmax,
                    )
                return last

            if trigger_mode == "split":
                # Two chunks: both preps early, so chunk 1's desc-gen runs
                # during the loads instead of gating trigger 1. Trigger 0
                # (count=1) fires prep 0's ring entry; its inherited deps
                # are backward-looking, and chunk 1's mins are emitted
                # AFTER it, so it waits only chunk 0's min. Trigger 1
                # (count=1, ring FIFO -> prep 1's entry) gets its data dep
                # on chunk 1's last min explicitly (not elidable: that min
                # is not covered by trigger 0's clock).
                preps = []
                for k in range(len(stores)):
                    p = emit_prep(k)
                    if preps:
                        # pin the ring FIFO order — without this edge Tile
                        # may reorder the (independent) preps and trigger k
                        # would fire the WRONG chunk's entry
                        add_dep_helper(p.ins, preps[-1].ins, sync=False)
                    preps.append(p)
                nc.vector.reduce_max(
                    out=wmax, in_=wmt[:, :OUT], axis=mybir.AxisListType.X
                )
                # chunk 0's mins precede trigger 0, so its inherited
                # (backward-looking) deps cover exactly min0. Trigger 0
                # also inherits a wait on EVERY pending prep's engine tick
                # (it could legally fire any of them) — the explicit
                # wait_ge(prep_sem, k+1) before each trigger provides the
                # correct per-entry desc-gen gate, and patch_trig0_prep_wait
                # drops the over-broad inherited one.
                emit_mins(0)
                prev = nc.gpsimd.trigger_dma(count=1)
                for k in range(1, len(stores)):
                    mk = emit_mins(k)
                    tk = nc.gpsimd.trigger_dma(count=1)
                    add_dep_helper(tk.ins, mk.ins, sync=True)
                    add_dep_helper(tk.ins, prev.ins, sync=False)
                    prev = tk
            elif trigger_mode == "single":
                # all preps early; one trigger inherits every prep's
                # deferred source dep, waits the last min, then fires all
                # ring entries back-to-back
                for k in range(len(stores)):
                    emit_prep(k)
                nc.vector.reduce_max(
                    out=wmax, in_=wmt[:, :OUT], axis=mybir.AxisListType.X
                )
                for k in range(len(stores)):
                    emit_mins(k)
                nc.gpsimd.trigger_dma(count=None)
            else:
                # paired: trigger k fires right after chunk k's min. Each
                # trigger_dma(count=None) consumes exactly its own prep's
                # pending entry, so its inherited dep is just that chunk's
                # min. prep k+1's desc-gen runs behind trigger k on
                # Pool.SEQ (~1us), which gates trigger k+1 — keep chunk
                # counts low.
                emit_prep(0)
                nc.vector.reduce_max(
                    out=wmax, in_=wmt[:, :OUT], axis=mybir.AxisListType.X
                )
                emit_mins(0)
                nc.gpsimd.trigger_dma(count=None)
                for k in range(1, len(stores)):
                    emit_prep(k)
                    emit_mins(k)
                    nc.gpsimd.trigger_dma(count=None)

    return nc


def patch_orphan_dmasw_waits(nc):
    """gen_mode==1 SWDGE preps defer their DMA-completion sem (on_update[0],
    baked into the descriptor at prep time) to trigger time, but Tile's wait
    pass still emits consumer waits against the prep's round-robin DMASW
    lane sem, which nothing increments. Rewrite those orphaned waits to the
    prep's real completion sem (same >=16 semantics)."""
    fn = nc.m.functions[0]
    insts = [i for b in fn.blocks for i in b.instructions]
    incs = {}
    for inst in insts:
        si = inst.sync_info
        if not si:
            continue
        for u in si.on_update:
            incs[u.id] = incs.get(u.id, 0) + (u.update_value or 0)
    lane_sem = {}
    k = 0
    for inst in insts:
        tn = type(inst).__name__
        if inst.engine == mybir.EngineType.Pool and (
            "DMACopy" in tn or "Gather" in tn or "Scatter" in tn
            or "Writeback" in tn or "RemoteDMA" in tn
        ):
            lane = k % 8
            k += 1
            if getattr(inst, "gen_mode", 0) == 1:
                si = inst.sync_info
                assert si and len(si.on_update) >= 1
                u0 = si.on_update[0]
                assert lane not in lane_sem, "one prep per DMASW lane"
                lane_sem[lane] = (u0.id, u0.ant_name)
    n = 0
    for inst in insts:
        si = inst.sync_info
        if not si:
            continue
        for w in si.on_wait:
            if (
                w.ant_name
                and w.ant_name.startswith("DMASW")
                and incs.get(w.id, 0) < (w.wait_value or 0)
            ):
                lane = int(w.ant_name[5:].split("_")[0])
                nid, nname = lane_sem[lane]
                w.id = nid
                w.ant_name = nname
                n += 1
    return n


def patch_split_war_waits(nc):
    """trigger_mode="split" emits chunk 1's mins AFTER the trigger that
    consumed prep 1's pending entry, so Tile adds a WAR wait on the mins:
    writer-of-otile1 waits the prep's deferred read = the s1 DMA completion
    sem — circular (that DMA fires only after the min). The hazard it
    guards is already covered: trigger 1 has an explicit sync dep on the
    last min, so the scatter can never read otile1 before the mins wrote
    it. Drop the circular wait (DVE instructions only; the final drains'
    completion waits on the same sems must stay)."""
    fn = nc.m.functions[0]
    n = 0
    for b in fn.blocks:
        for inst in b.instructions:
            if inst.engine != mybir.EngineType.DVE:
                continue
            si = inst.sync_info
            if not si:
                continue
            for w in si.on_wait:
                if (w.ant_name or "").startswith("sc_dma") and (
                    w.wait_value or 0
                ) > 0:
                    w.wait_value = 0
                    n += 1
    return n


def patch_trig0_prep_wait(nc):
    """Drop the first trigger's inherited wait on the Pool engine sem (it
    covers ALL pending preps' desc-gen ticks; the per-trigger
    wait_ge(sc_prep, k+1) already gates each trigger on exactly its own
    prep's descriptor-write completion)."""
    fn = nc.m.functions[0]
    insts = [i for b in fn.blocks for i in b.instructions]
    # only when the explicit sc_prep gate exists (split mode emits
    # wait_ge(sc_prep, k+1) before every trigger); NOTE: dropping this wait
    # crashed real HW in the 3-chunk test — keep it disabled until the
    # explicit gate is proven equivalent on silicon.
    if not any(
        (w.ant_name or "") == "sc_prep"
        for i in insts
        if i.sync_info
        for w in i.sync_info.on_wait
    ):
        return 0
    return 0


_NC_CACHE = {}


def _get_nc(**kwargs):
    key = repr(sorted(kwargs.items()))
    if key not in _NC_CACHE:
        nc = build_bass(**kwargs)
        nc.finalize()
        patch_orphan_dmasw_waits(nc)
        patch_split_war_waits(nc)
        patch_trig0_prep_wait(nc)
        _NC_CACHE[key] = nc
    return _NC_CACHE[key]


def shard_inputs(m, weight, wm_mcols=DEFAULT["wm_mcols"]):
    """Host-side staging: cast to bf16, transpose m so IN is the partition
    axis, fold the batch axis into partitions (2 halves), shard by IN rows.
    The first wm_mcols m columns are concatenated onto the weight tile."""
    bf16 = ml_dtypes.bfloat16
    mT = np.asarray(m, dtype=bf16).T  # [IN, B]
    wb = np.asarray(weight, dtype=bf16)  # [IN, OUT]
    half = B // 2
    in_maps = []
    for c in range(NCORES):
        rows = mT[c * RPC : (c + 1) * RPC]  # [64, B]
        m_c = np.concatenate([rows[:, :half], rows[:, half:]], axis=0)
        w_c = np.tile(wb[c * RPC : (c + 1) * RPC, :], (2, 1))  # [128, 256]
        io = {
            "wm": np.ascontiguousarray(
                np.concatenate([w_c, m_c[:, :wm_mcols]], axis=1)
            )
        }
        if wm_mcols < COLS:
            io["m"] = np.ascontiguousarray(m_c[:, wm_mcols:])
        in_maps.append(io)
    return in_maps


def unshard_output(results):
    """Per-core [OUT_ROWS_PAD, 2048] bf16 (first 128 rows valid) ->
    full [B, IN] f32."""
    half = B // 2
    outT = np.empty((IN, B), dtype=np.float32)
    for c in range(NCORES):
        o_c = np.asarray(results[c]["out"])[:P]  # [128, 2048] bf16
        outT[c * RPC : (c + 1) * RPC, :half] = o_c[:RPC]
        outT[c * RPC : (c + 1) * RPC, half:] = o_c[RPC:]
    return np.ascontiguousarray(outT.T)


def run(m, weight, build_kwargs=None, **spmd_kwargs):
    """Run the bass kernel; returns (full_output, BassKernelResults)."""
    bk = dict(DEFAULT)
    bk.update(build_kwargs or {})
    nc = _get_nc(**bk)
    in_maps = shard_inputs(m, weight, wm_mcols=bk["wm_mcols"])
    res = run_bass_kernel_spmd(nc, in_maps, list(range(NCORES)), **spmd_kwargs)
    return unshard_output(res.results), res


def kernel(m, weight):
    return run(m, weight)[0]
